# revision 53
# baseline (speedup 1.0000x reference)
"""ArcFace multi-head-sharded loss on 8 TRN2 NeuronCores.

Strategy: shard the (64, 2048, 256) weight table over the group axis —
each core owns 8 groups. Samples are routed host-side to the core owning
their group (host routing replaces the all-to-all). Weight rows are
l2-normalized host-side and quantized to fp8e4 (x16 pre-scale to stay in
the normal range); embeddings stay bf16 (PE runs mixed bf16 x fp8 at
full column rate). The device does:

  - stream its 8 weight groups (4MB fp8) from HBM on the sync HWDGE ring
    (a single ring sustains ~366 GB/s; the last band streams as 4
    per-chunk pieces so the tail unlocks incrementally),
  - mains: psum(b, c) = <x_b, wq_c> on PE. Four 32-sample bands sit in
    the four column quadrants of the array (tile_position); the j-loop
    is innermost within each (chunk, k) group so the four quadrant
    matmuls overlap (~4ns issue stagger),
  - exp per 512-class chunk with the class-axis sum fused via accum_out
    (scale folds the 1/16 weight quantization scale and the ArcFace
    scale 64),
  - lb = ln(a*sum + b) where (a, b) fold the entire ArcFace margin +
    target-logit swap, precomputed host-side per sample,
  - one [128,1] f32 column out per sample tile (first tile's output DMA
    hides under the weight stream).

Host: sums the lb of valid rows across cores, /B + SHIFT. ~4MB HBM
traffic per core => memory-bound.

Samples are packed into "bands" of NG=32 partition rows, one band per
weight group (plus overflow bands), 4 bands per 128-row sample tile.
"""

import sys
import numpy as np
import ml_dtypes

BF16 = ml_dtypes.bfloat16
FP8 = ml_dtypes.float8_e4m3

_TRN_REPO = "/opt/trn_rl_repo"
if _TRN_REPO not in sys.path:
    sys.path.insert(0, _TRN_REPO)

# problem config (hardcoded per spec)
B, E, G, C = 512, 256, 64, 2048
NCORES = 8
GPC = G // NCORES        # weight groups per core
NG = 32                  # sample slots per band
BPT = 128 // NG          # bands per 128-partition sample tile
KE = E // 128            # contraction chunks
NCC = C // 512           # 512-col chunks per group
SCALE = 64.0
MARGIN = 0.5
COS_M = float(np.cos(MARGIN))
SIN_M = float(np.sin(MARGIN))
THETA = float(np.cos(np.pi - MARGIN))
SINMM = float(np.sin(np.pi - MARGIN) * MARGIN)
EPS = 1e-12
WS = 16.0                # fp8 weight pre-scale; PSUM = WS * cos (x is bf16)
ESCALE = SCALE / WS
LB_SHIFT = float(40.0 * np.log(2.0))  # ln-range shift, re-added host-side

_graph_cache = {}


def _build(nb):
    """Build the per-core Bass graph for nb weight bands (nb % BPT == 0)."""
    from contextlib import ExitStack
    import concourse.bacc as bacc
    import concourse.tile as tile
    from concourse import mybir

    f32 = mybir.dt.float32
    bf16 = mybir.dt.bfloat16
    fp8 = mybir.dt.float8e4
    AF = mybir.ActivationFunctionType

    T = nb // BPT
    npair = (nb - 2) // 2  # bands 0..nb-3 stream as 1MB pairs
    nc = bacc.Bacc(None)

    # bands 0..nb-2 in one flat param (band-major [KE, C] per partition);
    # the last band is piece-major so each 512-class piece is contiguous
    wt_ext = nc.declare_dram_parameter("wt", [128, nb - 1, KE, C], fp8,
                                       isOutput=False)
    wt7_ext = nc.declare_dram_parameter("wt7", [128, NCC, KE, 512], fp8,
                                        isOutput=False)
    xtq_ext = nc.declare_dram_parameter("xtq", [128, T, KE, 128], bf16,
                                        isOutput=False)
    ab_ext = nc.declare_dram_parameter("ab", [128, 2 * T], f32, isOutput=False)
    out_ext = nc.declare_dram_parameter("out", [128, T], f32, isOutput=True)

    with tile.TileContext(nc) as tc, ExitStack() as ctx:
        wpool = ctx.enter_context(tc.tile_pool(name="w", bufs=npair + 1))
        ppool = ctx.enter_context(tc.tile_pool(name="wp", bufs=NCC))
        cpool = ctx.enter_context(tc.tile_pool(name="const", bufs=1))
        vpool = ctx.enter_context(tc.tile_pool(name="vec", bufs=2))
        pmain = ctx.enter_context(tc.tile_pool(name="pmain", bufs=8, space="PSUM"))

        g_tiles = [wpool.tile([128, 2, KE, C], fp8, tag="wt", name=f"wt{i}")
                   for i in range(npair)]
        b6_tile = wpool.tile([128, KE, C], fp8, tag="wt", name="wt6")
        p_tiles = [ppool.tile([128, KE, 512], fp8, tag="wp", name=f"wp{cc}")
                   for cc in range(NCC)]
        xtq_sb = cpool.tile([128, T, KE, 128], bf16, tag="xtq")
        ab_sb = cpool.tile([128, 2 * T], f32, tag="ab")
        lb_sb = cpool.tile([128, T], f32, tag="lb")

        def rhs_ap(b, k, cc):
            if b == nb - 1:
                return p_tiles[cc][:, k, :]
            if b == nb - 2:
                return b6_tile[:, k, 512 * cc: 512 * cc + 512]
            return g_tiles[b // 2][:, b % 2, k, 512 * cc: 512 * cc + 512]

        # tiny inputs ride the scalar HWDGE ring so the weight stream on
        # the sync ring is never interrupted
        nc.scalar.dma_start(out=xtq_sb[:], in_=xtq_ext[:])
        nc.scalar.dma_start(out=ab_sb[:], in_=ab_ext[:])

        # preload the natural_log_exp_and_others ACT table set (exp, ln)
        # early so the table DMA hides under the weight stream
        nc.scalar.add_instruction(mybir.InstLoadActFuncSet(
            name="preload-actset-6", act_func_set_id=6, ins=[], outs=[]))

        # the weight stream: pairs, then the single band nb-2, then the
        # last band as 4 per-chunk pieces (tail unlocks incrementally)
        for i in range(npair):
            nc.sync.dma_start(out=g_tiles[i][:], in_=wt_ext[:, 2 * i:2 * i + 2])
        nc.sync.dma_start(out=b6_tile[:], in_=wt_ext[:, nb - 2])
        for cc in range(NCC):
            nc.sync.dma_start(out=p_tiles[cc][:], in_=wt7_ext[:, cc])

        # exp scale as a per-partition AP (matches the fast ACT path; an
        # immediate scale measured ~2x slower per column)
        escale = cpool.tile([128, 1], f32, tag="escale")
        nc.vector.memset(escale[:], ESCALE)

        cps_t = {t: [pmain.tile([128, 512], f32, tag="cos", name=f"cos{t}_{cc}")
                     for cc in range(NCC)] for t in range(T)}
        ses_t = {t: cpool.tile([128, NCC], f32, tag=f"ses{t}", name=f"ses{t}")
                 for t in range(T)}

        def mm(t, cc, k, j):
            nc.tensor.matmul(
                cps_t[t][cc][NG * j:NG * (j + 1), :],
                xtq_sb[:, t, k, NG * j:NG * (j + 1)],
                rhs_ap(BPT * t + j, k, cc),
                start=(k == 0), stop=(k == KE - 1),
                tile_position=(0, NG * j),
            )

        def emit_exp(t, cc):
            """PSUM chunk -> bf16 SBUF via DVE (frees the bank, feeds ACT
            its fast input path), then exp with the class-axis sum fused
            via accum_out."""
            cb = cpool.tile([128, 512], bf16, tag=f"cosbf{t}_{cc}",
                            name=f"cosbf{t}_{cc}")
            nc.vector.tensor_copy(cb[:], cps_t[t][cc][:])
            escr = vpool.tile([128, 512], bf16, tag="escr")
            nc.scalar.activation(escr[:], cb[:], AF.Exp, scale=escale[:],
                                 accum_out=ses_t[t][:, cc:cc + 1])

        def emit_tail(t):
            """lb = ln(a * sum_cc ses + b); (a, b) fold margin + swap."""
            sfull = cpool.tile([128, 1], f32, tag=f"sfull{t}")
            nc.vector.reduce_sum(sfull[:], ses_t[t][:], axis=mybir.AxisListType.X)
            nc.scalar.activation(lb_sb[:, t:t + 1], sfull[:], AF.Ln,
                                 scale=ab_sb[:, t:t + 1],
                                 bias=ab_sb[:, T + t:T + t + 1])
            nc.scalar.dma_start(out=out_ext[:, t:t + 1], in_=lb_sb[:, t:t + 1])

        for t in range(T):
            for cc in range(NCC):
                # j innermost within each (cc, k) so the quadrant matmuls
                # overlap on distinct PE column strips (~4ns stagger).
                # The last tile's j=3 rides inside the per-chunk loop: its
                # per-chunk weight pieces land staggered, so each chunk
                # completes (and exps) as soon as its piece arrives instead
                # of queueing behind all other bands' matmuls.
                for k in range(KE):
                    for j in range(BPT):
                        mm(t, cc, k, j)
                emit_exp(t, cc)
            emit_tail(t)

    nc.compile()
    return nc


def _pack(logits, labels, weight):
    """Route samples to the core owning their group; build per-core inputs."""
    logits = np.asarray(logits, dtype=np.float32)
    labels = np.asarray(labels).astype(np.int64)
    weight = np.asarray(weight, dtype=np.float32)

    group = (labels // C).astype(np.int64)
    local = (labels % C).astype(np.int64)
    core = group // GPC
    gl = group % GPC

    # host-side l2 normalization; weights quantized to fp8 (x16 keeps
    # values in fp8e4's normal range; cos is invariant to row scaling),
    # x stays bf16 (PE runs mixed bf16 x fp8 at the same column rate)
    xn = logits / np.maximum(
        np.sqrt(np.sum(logits * logits, axis=1, keepdims=True)), EPS)
    wn2 = np.sqrt(np.einsum("gce,gce->gc", weight, weight))[:, :, None]
    wn = weight / np.maximum(wn2, EPS)
    wq = (WS * wn).astype(FP8)                    # (G, C, E) fp8 table
    xb = xn.astype(BF16)                          # (B, E) bf16

    # per-sample margin + swap folded into (a, b):
    #   lb = ln(a * sum_c exp(ESCALE*psum_c) + b)
    #      = -64*ft - SHIFT + ln(sumexp with target swapped to 64*ft)
    # s_mm mimics the device's value of the target column: bf16-x times
    # fp8-w dot in f32, then the bf16 rounding of the PSUM->SBUF copy.
    wq_tar = wq[group, local].astype(np.float32)            # (B, E)
    psum_h = np.einsum("be,be->b", xb.astype(np.float32), wq_tar)
    s_mm = psum_h.astype(BF16).astype(np.float64) * ESCALE  # = 64*t_mm
    t = np.einsum("be,be->b", xn, wn[group, local]).astype(np.float64)
    sin_t = np.sqrt(np.clip(1.0 - t * t, 0.0, None))
    ft = np.where(t > THETA, t * COS_M - sin_t * SIN_M, t - SINMM)
    ft = np.where(labels != -1, ft, t)
    a64 = np.exp(-SCALE * ft - LB_SHIFT)
    b64 = (np.exp(SCALE * ft) - np.exp(s_mm)) * a64
    a32 = a64.astype(np.float32)
    b32 = b64.astype(np.float32)

    # band assignment: per (core, local-group), ceil(count/NG) bands
    percg = [[np.nonzero((core == c) & (gl == g))[0] for g in range(GPC)]
             for c in range(NCORES)]
    nbands = [sum(max(1, -(-len(idx) // NG)) for idx in percg[c])
              for c in range(NCORES)]
    nb = max(nbands)
    nb = -(-nb // BPT) * BPT  # round up to full sample tiles
    T = nb // BPT

    in_maps = []
    valid_rows = []
    for c in range(NCORES):
        # band -> (group, sample indices)
        bands = []
        for g in range(GPC):
            idx = percg[c][g]
            nslice = max(1, -(-len(idx) // NG))
            for s in range(nslice):
                bands.append((g, idx[s * NG:(s + 1) * NG]))
        while len(bands) < nb:
            bands.append((0, np.empty(0, dtype=np.int64)))

        xbp = np.zeros((T, 128, E), dtype=BF16)
        ab = np.ones((128, 2 * T), dtype=np.float32)
        valid = np.zeros((128, T), dtype=bool)
        wt = np.empty((128, nb - 1, KE, C), dtype=FP8)
        wt7 = np.empty((128, NCC, KE, 512), dtype=FP8)
        for b, (g, idx) in enumerate(bands):
            wg = wq[c * GPC + g]                     # (C, E) fp8
            # warr[p, k, c] = wg[c, 128k+p]
            warr = np.ascontiguousarray(wg.reshape(C, KE, 128).transpose(2, 1, 0))
            if b < nb - 1:
                wt[:, b] = warr
            else:
                for cc in range(NCC):
                    wt7[:, cc] = warr[:, :, 512 * cc:512 * cc + 512]
            ti, j = b // BPT, b % BPT
            sl = slice(NG * j, NG * j + len(idx))
            xbp[ti, sl, :] = xb[idx]
            ab[sl, ti] = a32[idx]
            ab[sl, T + ti] = b32[idx]
            valid[sl, ti] = True
        # xtq[p, t, k, r] = xbp[t][r, 128k+p] (transposed PE stationary x)
        xtq = np.ascontiguousarray(np.transpose(
            xbp.reshape(T, 128, KE, 128), (3, 0, 2, 1)))
        in_maps.append({"wt": wt, "wt7": wt7, "xtq": xtq, "ab": ab})
        valid_rows.append(valid)
    return in_maps, nb, valid_rows


def _run(logits, labels, weight, trace=False, **kw):
    from concourse.bass_utils import run_bass_kernel_spmd

    in_maps, nb, valid_rows = _pack(logits, labels, weight)
    nc = _graph_cache.get(nb)
    if nc is None:
        nc = _build(nb)
        _graph_cache[nb] = nc
    res = run_bass_kernel_spmd(nc, in_maps, core_ids=list(range(NCORES)),
                               trace=trace, **kw)
    total = sum(
        float(np.asarray(res.results[i]["out"], dtype=np.float32)[valid_rows[i]].sum())
        for i in range(NCORES)) / B + LB_SHIFT
    return np.asarray(total, dtype=np.float32), res


def kernel(logits, labels, weight):
    loss, _ = _run(logits, labels, weight)
    return loss


# revision 54
# speedup vs baseline: 1.0124x; 1.0124x over previous
"""ArcFace multi-head-sharded loss on 8 TRN2 NeuronCores.

Strategy: shard the (64, 2048, 256) weight table over the group axis —
each core owns 8 groups. Samples are routed host-side to the core owning
their group (host routing replaces the all-to-all). Weight rows are
l2-normalized host-side and quantized to fp8e4 (x16 pre-scale to stay in
the normal range); embeddings stay bf16 (PE runs mixed bf16 x fp8 at
full column rate). The device does:

  - stream its 8 weight groups (4MB fp8) from HBM on the sync HWDGE ring
    (a single ring sustains ~366 GB/s; the last band streams as 4
    per-chunk pieces so the tail unlocks incrementally),
  - mains: psum(b, c) = <x_b, wq_c> on PE. Four 32-sample bands sit in
    the four column quadrants of the array (tile_position); the j-loop
    is innermost within each (chunk, k) group so the four quadrant
    matmuls overlap (~4ns issue stagger),
  - exp per 512-class chunk with the class-axis sum fused via accum_out
    (scale folds the 1/16 weight quantization scale and the ArcFace
    scale 64),
  - lb = ln(a*sum + b) where (a, b) fold the entire ArcFace margin +
    target-logit swap, precomputed host-side per sample,
  - one [128,1] f32 column out per sample tile (first tile's output DMA
    hides under the weight stream).

Host: sums the lb of valid rows across cores, /B + SHIFT. ~4MB HBM
traffic per core => memory-bound.

Samples are packed into "bands" of NG=32 partition rows, one band per
weight group (plus overflow bands), 4 bands per 128-row sample tile.
"""

import sys
import numpy as np
import ml_dtypes

BF16 = ml_dtypes.bfloat16
FP8 = ml_dtypes.float8_e4m3

_TRN_REPO = "/opt/trn_rl_repo"
if _TRN_REPO not in sys.path:
    sys.path.insert(0, _TRN_REPO)

# problem config (hardcoded per spec)
B, E, G, C = 512, 256, 64, 2048
NCORES = 8
GPC = G // NCORES        # weight groups per core
NG = 32                  # sample slots per band
BPT = 128 // NG          # bands per 128-partition sample tile
KE = E // 128            # contraction chunks
NCC = C // 512           # 512-col chunks per group
SCALE = 64.0
MARGIN = 0.5
COS_M = float(np.cos(MARGIN))
SIN_M = float(np.sin(MARGIN))
THETA = float(np.cos(np.pi - MARGIN))
SINMM = float(np.sin(np.pi - MARGIN) * MARGIN)
EPS = 1e-12
WS = 16.0                # fp8 weight pre-scale; PSUM = WS * cos (x is bf16)
ESCALE = SCALE / WS
LB_SHIFT = float(40.0 * np.log(2.0))  # ln-range shift, re-added host-side

_graph_cache = {}


def _build(nb):
    """Build the per-core Bass graph for nb weight bands (nb % BPT == 0)."""
    from contextlib import ExitStack
    import concourse.bacc as bacc
    import concourse.tile as tile
    from concourse import mybir

    f32 = mybir.dt.float32
    bf16 = mybir.dt.bfloat16
    fp8 = mybir.dt.float8e4
    AF = mybir.ActivationFunctionType

    T = nb // BPT
    npair = (nb - 2) // 2  # bands 0..nb-3 stream as 1MB pairs
    nc = bacc.Bacc(None)

    # bands 0..nb-2 in one flat param (band-major [KE, C] per partition);
    # the last band is piece-major so each 512-class piece is contiguous
    wt_ext = nc.declare_dram_parameter("wt", [128, nb - 1, KE, C], fp8,
                                       isOutput=False)
    wt7_ext = nc.declare_dram_parameter("wt7", [128, NCC, KE, 512], fp8,
                                        isOutput=False)
    xtq_ext = nc.declare_dram_parameter("xtq", [128, T, KE, 128], bf16,
                                        isOutput=False)
    ab_ext = nc.declare_dram_parameter("ab", [128, 2 * T], f32, isOutput=False)
    out_ext = nc.declare_dram_parameter("out", [128, T], f32, isOutput=True)

    with tile.TileContext(nc) as tc, ExitStack() as ctx:
        wpool = ctx.enter_context(tc.tile_pool(name="w", bufs=npair + 1))
        ppool = ctx.enter_context(tc.tile_pool(name="wp", bufs=NCC))
        cpool = ctx.enter_context(tc.tile_pool(name="const", bufs=1))
        vpool = ctx.enter_context(tc.tile_pool(name="vec", bufs=2))
        pmain = ctx.enter_context(tc.tile_pool(name="pmain", bufs=8, space="PSUM"))

        g_tiles = [wpool.tile([128, 2, KE, C], fp8, tag="wt", name=f"wt{i}")
                   for i in range(npair)]
        b6_tile = wpool.tile([128, KE, C], fp8, tag="wt", name="wt6")
        p_tiles = [ppool.tile([128, KE, 512], fp8, tag="wp", name=f"wp{cc}")
                   for cc in range(NCC)]
        xtq_sb = cpool.tile([128, T, KE, 128], bf16, tag="xtq")
        ab_sb = cpool.tile([128, 2 * T], f32, tag="ab")
        lb_sb = cpool.tile([128, T], f32, tag="lb")

        def rhs_ap(b, k, cc):
            if b == nb - 1:
                return p_tiles[cc][:, k, :]
            if b == nb - 2:
                return b6_tile[:, k, 512 * cc: 512 * cc + 512]
            return g_tiles[b // 2][:, b % 2, k, 512 * cc: 512 * cc + 512]

        # tiny inputs ride the scalar HWDGE ring so the weight stream on
        # the sync ring is never interrupted
        nc.scalar.dma_start(out=xtq_sb[:], in_=xtq_ext[:])
        nc.scalar.dma_start(out=ab_sb[:], in_=ab_ext[:])

        # preload the natural_log_exp_and_others ACT table set (exp, ln)
        # early so the table DMA hides under the weight stream
        nc.scalar.add_instruction(mybir.InstLoadActFuncSet(
            name="preload-actset-6", act_func_set_id=6, ins=[], outs=[]))

        # the weight stream: pairs, then the single band nb-2, then the
        # last band as 4 per-chunk pieces (tail unlocks incrementally)
        for i in range(npair):
            nc.sync.dma_start(out=g_tiles[i][:], in_=wt_ext[:, 2 * i:2 * i + 2])
        nc.sync.dma_start(out=b6_tile[:], in_=wt_ext[:, nb - 2])
        for cc in range(NCC):
            nc.sync.dma_start(out=p_tiles[cc][:], in_=wt7_ext[:, cc])

        # exp scale as a per-partition AP (matches the fast ACT path; an
        # immediate scale measured ~2x slower per column)
        escale = cpool.tile([128, 1], f32, tag="escale")
        nc.vector.memset(escale[:], ESCALE)

        # ACT keep-alive: the engine pays a 1-3us semaphore-wake penalty
        # when it idles on a wait registered long before the producer
        # finishes (measured repeatedly). Fill its otherwise-idle window
        # (table preload ~9.6us -> first chunk ready ~15us) with cheap
        # dummy exps so the first real exp's wait is satisfied on arrival.
        dscr = cpool.tile([128, 1], f32, tag="dscr")
        for _ in range(16):
            nc.scalar.activation(dscr[:], escale[:], AF.Exp)

        cps_t = {t: [pmain.tile([128, 512], f32, tag="cos", name=f"cos{t}_{cc}")
                     for cc in range(NCC)] for t in range(T)}
        ses_t = {t: cpool.tile([128, NCC], f32, tag=f"ses{t}", name=f"ses{t}")
                 for t in range(T)}

        def mm(t, cc, k, j):
            nc.tensor.matmul(
                cps_t[t][cc][NG * j:NG * (j + 1), :],
                xtq_sb[:, t, k, NG * j:NG * (j + 1)],
                rhs_ap(BPT * t + j, k, cc),
                start=(k == 0), stop=(k == KE - 1),
                tile_position=(0, NG * j),
            )

        def emit_exp(t, cc):
            """PSUM chunk -> bf16 SBUF via DVE (frees the bank, feeds ACT
            its fast input path), then exp with the class-axis sum fused
            via accum_out."""
            cb = cpool.tile([128, 512], bf16, tag=f"cosbf{t}_{cc}",
                            name=f"cosbf{t}_{cc}")
            nc.vector.tensor_copy(cb[:], cps_t[t][cc][:])
            escr = vpool.tile([128, 512], bf16, tag="escr")
            nc.scalar.activation(escr[:], cb[:], AF.Exp, scale=escale[:],
                                 accum_out=ses_t[t][:, cc:cc + 1])

        def emit_tail(t):
            """lb = ln(a * sum_cc ses + b); (a, b) fold margin + swap."""
            sfull = cpool.tile([128, 1], f32, tag=f"sfull{t}")
            nc.vector.reduce_sum(sfull[:], ses_t[t][:], axis=mybir.AxisListType.X)
            nc.scalar.activation(lb_sb[:, t:t + 1], sfull[:], AF.Ln,
                                 scale=ab_sb[:, t:t + 1],
                                 bias=ab_sb[:, T + t:T + t + 1])
            nc.scalar.dma_start(out=out_ext[:, t:t + 1], in_=lb_sb[:, t:t + 1])

        for t in range(T):
            for cc in range(NCC):
                # j innermost within each (cc, k) so the quadrant matmuls
                # overlap on distinct PE column strips (~4ns stagger).
                # The last tile's j=3 rides inside the per-chunk loop: its
                # per-chunk weight pieces land staggered, so each chunk
                # completes (and exps) as soon as its piece arrives instead
                # of queueing behind all other bands' matmuls.
                for k in range(KE):
                    for j in range(BPT):
                        mm(t, cc, k, j)
                emit_exp(t, cc)
            emit_tail(t)

    nc.compile()
    return nc


def _pack(logits, labels, weight):
    """Route samples to the core owning their group; build per-core inputs."""
    logits = np.asarray(logits, dtype=np.float32)
    labels = np.asarray(labels).astype(np.int64)
    weight = np.asarray(weight, dtype=np.float32)

    group = (labels // C).astype(np.int64)
    local = (labels % C).astype(np.int64)
    core = group // GPC
    gl = group % GPC

    # host-side l2 normalization; weights quantized to fp8 (x16 keeps
    # values in fp8e4's normal range; cos is invariant to row scaling),
    # x stays bf16 (PE runs mixed bf16 x fp8 at the same column rate)
    xn = logits / np.maximum(
        np.sqrt(np.sum(logits * logits, axis=1, keepdims=True)), EPS)
    wn2 = np.sqrt(np.einsum("gce,gce->gc", weight, weight))[:, :, None]
    wn = weight / np.maximum(wn2, EPS)
    wq = (WS * wn).astype(FP8)                    # (G, C, E) fp8 table
    xb = xn.astype(BF16)                          # (B, E) bf16

    # per-sample margin + swap folded into (a, b):
    #   lb = ln(a * sum_c exp(ESCALE*psum_c) + b)
    #      = -64*ft - SHIFT + ln(sumexp with target swapped to 64*ft)
    # s_mm mimics the device's value of the target column: bf16-x times
    # fp8-w dot in f32, then the bf16 rounding of the PSUM->SBUF copy.
    wq_tar = wq[group, local].astype(np.float32)            # (B, E)
    psum_h = np.einsum("be,be->b", xb.astype(np.float32), wq_tar)
    s_mm = psum_h.astype(BF16).astype(np.float64) * ESCALE  # = 64*t_mm
    t = np.einsum("be,be->b", xn, wn[group, local]).astype(np.float64)
    sin_t = np.sqrt(np.clip(1.0 - t * t, 0.0, None))
    ft = np.where(t > THETA, t * COS_M - sin_t * SIN_M, t - SINMM)
    ft = np.where(labels != -1, ft, t)
    a64 = np.exp(-SCALE * ft - LB_SHIFT)
    b64 = (np.exp(SCALE * ft) - np.exp(s_mm)) * a64
    a32 = a64.astype(np.float32)
    b32 = b64.astype(np.float32)

    # band assignment: per (core, local-group), ceil(count/NG) bands
    percg = [[np.nonzero((core == c) & (gl == g))[0] for g in range(GPC)]
             for c in range(NCORES)]
    nbands = [sum(max(1, -(-len(idx) // NG)) for idx in percg[c])
              for c in range(NCORES)]
    nb = max(nbands)
    nb = -(-nb // BPT) * BPT  # round up to full sample tiles
    T = nb // BPT

    in_maps = []
    valid_rows = []
    for c in range(NCORES):
        # band -> (group, sample indices)
        bands = []
        for g in range(GPC):
            idx = percg[c][g]
            nslice = max(1, -(-len(idx) // NG))
            for s in range(nslice):
                bands.append((g, idx[s * NG:(s + 1) * NG]))
        while len(bands) < nb:
            bands.append((0, np.empty(0, dtype=np.int64)))

        xbp = np.zeros((T, 128, E), dtype=BF16)
        ab = np.ones((128, 2 * T), dtype=np.float32)
        valid = np.zeros((128, T), dtype=bool)
        wt = np.empty((128, nb - 1, KE, C), dtype=FP8)
        wt7 = np.empty((128, NCC, KE, 512), dtype=FP8)
        for b, (g, idx) in enumerate(bands):
            wg = wq[c * GPC + g]                     # (C, E) fp8
            # warr[p, k, c] = wg[c, 128k+p]
            warr = np.ascontiguousarray(wg.reshape(C, KE, 128).transpose(2, 1, 0))
            if b < nb - 1:
                wt[:, b] = warr
            else:
                for cc in range(NCC):
                    wt7[:, cc] = warr[:, :, 512 * cc:512 * cc + 512]
            ti, j = b // BPT, b % BPT
            sl = slice(NG * j, NG * j + len(idx))
            xbp[ti, sl, :] = xb[idx]
            ab[sl, ti] = a32[idx]
            ab[sl, T + ti] = b32[idx]
            valid[sl, ti] = True
        # xtq[p, t, k, r] = xbp[t][r, 128k+p] (transposed PE stationary x)
        xtq = np.ascontiguousarray(np.transpose(
            xbp.reshape(T, 128, KE, 128), (3, 0, 2, 1)))
        in_maps.append({"wt": wt, "wt7": wt7, "xtq": xtq, "ab": ab})
        valid_rows.append(valid)
    return in_maps, nb, valid_rows


def _run(logits, labels, weight, trace=False, **kw):
    from concourse.bass_utils import run_bass_kernel_spmd

    in_maps, nb, valid_rows = _pack(logits, labels, weight)
    nc = _graph_cache.get(nb)
    if nc is None:
        nc = _build(nb)
        _graph_cache[nb] = nc
    res = run_bass_kernel_spmd(nc, in_maps, core_ids=list(range(NCORES)),
                               trace=trace, **kw)
    total = sum(
        float(np.asarray(res.results[i]["out"], dtype=np.float32)[valid_rows[i]].sum())
        for i in range(NCORES)) / B + LB_SHIFT
    return np.asarray(total, dtype=np.float32), res


def kernel(logits, labels, weight):
    loss, _ = _run(logits, labels, weight)
    return loss
